# revision 35
# baseline (speedup 1.0000x reference)
"""LiquidCell Trainium2 kernel (Bass/Tile, 8-core SPMD, data-parallel over batch).

Reference computation (B=4096, I=1024, H=2048, 5 steps):
    input_contrib = x @ W_in_w.T + W_in_b
    x_tau = x @ tau_adapt_w[:, :I].T
    h = hidden
    for _ in range(5):
        tau_logits = x_tau + h @ tau_adapt_w[:, I:].T + tau_adapt_b
        tau = tau_base * (0.5 + sigmoid(tau_logits))
        activated = tanh(h @ W_rec.T + input_contrib)
        h = h + DT * (-h + activated) / tau
    return (h, tau)

Strategy: shard batch across 8 cores (512 rows each), replicate weights.
Everything on-chip is feature-major ([features on partitions, batch cols
free]) so the recurrent state feeds matmuls without transposes.

Precision plan (validated against the fp32 reference in numpy, rel err
~8e-3 vs the 2e-2 gate):
  - preamble (x-side matmuls, computed once): bf16
  - recurrent-step matmuls (steps 0-4 W_rec path, steps 0-3 tau path):
    fp8 e4m3 in DoubleRow perf mode (2 k-tiles per instruction, 2x PE
    throughput). Weights are pre-scaled by a power-of-2 per-tensor scale
    on host so their magnitudes sit in e4m3's normal range; the descale
    rides the PSUM-readout activation for free. h is quantized to fp8
    once per step on the vector engine; the f32 master h is what the
    elementwise update integrates, so quantization error does not
    accumulate across steps.
  - last step's tau matmul: f32r (tau is an output; fp8 there pushes
    err_tau past the gate, f32r keeps it at ~4e-3)
Both fp8 weight tensors (8 MiB total) are SBUF-resident, loaded once, so
per-step HBM traffic is ~zero (the baseline streamed 32 MiB/step and was
DMA-bound at 95% occupancy). The f32r tau weights for the last step
stream during steps 3-4 on both HWDGE rings.

PSUM trick: the loop-invariant x_tau and input_contrib (both scaled)
are pre-loaded into the PSUM banks by the scalar engine and both
matmul chains run with start=False, accumulating on top - this removes
two vector adds per tile and keeps scalar/vector balanced against the
fp8-rate PE. (DVE has no divide ALU op on trn2; the update uses
reciprocal_approx_fast + multiply.)
"""

import os

import numpy as np
import ml_dtypes

import concourse.bacc as bacc
import concourse.mybir as mybir
import concourse.tile as tile
from concourse.bass_utils import run_bass_kernel_spmd

F32 = mybir.dt.float32
F32R = mybir.dt.float32r
BF16 = mybir.dt.bfloat16
F8 = mybir.dt.float8e4
AF = mybir.ActivationFunctionType
ALU = mybir.AluOpType
DR = mybir.MatmulPerfMode.DoubleRow

NP_BF16 = ml_dtypes.bfloat16
NP_F8 = ml_dtypes.float8_e4m3

B, I, H = 4096, 1024, 2048
NUM_STEPS = 5
DT = 0.1
NCORES = 8
BL = B // NCORES          # 512 batch rows per core
P = 128
JT = H // P               # 16 output-feature tiles
KTH = H // P              # 16 contraction tiles (h side)
KP = KTH // 2             # 8 DoubleRow k-pairs
KTX = I // P              # 8 contraction tiles (x side)

# exposed for test harness (set when BASS_TRACE=1)
LAST_EXEC_NS = None


def _build():
    nc = bacc.Bacc()
    xT_d = nc.declare_dram_parameter("xT", [I, BL], BF16, isOutput=False)
    hT_d = nc.declare_dram_parameter("hT", [H, BL], F32R, isOutput=False)
    h8T_d = nc.declare_dram_parameter("h8T", [H, BL], F8, isOutput=False)
    Wi_d = nc.declare_dram_parameter("Wi", [JT, P, KTX, P], BF16, isOutput=False)
    Tx_d = nc.declare_dram_parameter("Tx", [JT, P, KTX, P], BF16, isOutput=False)
    Wr8_d = nc.declare_dram_parameter("Wr8", [JT, P, KTH, P], F8, isOutput=False)
    Th8_d = nc.declare_dram_parameter("Th8", [JT, P, KTH, P], F8, isOutput=False)
    ThR_d = nc.declare_dram_parameter("ThR", [JT, P, KTH, P], F32R, isOutput=False)
    # per-feature vectors, laid out [P, JT] (col j = features j*128..j*128+127)
    taub_d = nc.declare_dram_parameter("taub", [P, JT], F32, isOutput=False)
    tbdt_d = nc.declare_dram_parameter("tbdt", [P, JT], F32, isOutput=False)
    htbdt_d = nc.declare_dram_parameter("htbdt", [P, JT], F32, isOutput=False)
    tb_d = nc.declare_dram_parameter("tb", [P, JT], F32, isOutput=False)
    htb_d = nc.declare_dram_parameter("htb", [P, JT], F32, isOutput=False)
    winbs_d = nc.declare_dram_parameter("winbs", [P, JT], F32, isOutput=False)
    scl_d = nc.declare_dram_parameter("scl", [P, 2], F32, isOutput=False)
    hout_d = nc.declare_dram_parameter("hout", [H, BL], F32, isOutput=True)
    tauout_d = nc.declare_dram_parameter("tauout", [H, BL], F32, isOutput=True)

    with tile.TileContext(nc) as tc:
        with tc.tile_pool(name="const", bufs=1) as const, \
             tc.tile_pool(name="big", bufs=1) as big, \
             tc.tile_pool(name="h8p", bufs=2) as h8p, \
             tc.tile_pool(name="wpre", bufs=4) as wpre, \
             tc.tile_pool(name="wstr", bufs=3) as wstr, \
             tc.tile_pool(name="sc", bufs=2) as sc, \
             tc.tile_pool(name="ps", bufs=4, space="PSUM") as ps:

            rings = (nc.scalar, nc.sync)

            # ---- cold-start DMAs; the two HWDGE rings carry the
            # latency-critical preamble stream, gpsimd SWDGE carries
            # state/consts whose first use is ~55us in ----
            pre_slabs = []
            txs = wpre.tile([P, KTX, P], BF16, tag="tx")
            nc.scalar.dma_start(out=txs, in_=Tx_d[0])
            wis = wpre.tile([P, KTX, P], BF16, tag="wi")
            nc.sync.dma_start(out=wis, in_=Wi_d[0])
            pre_slabs.append((txs, wis))
            # xT shares the "thr" slot rotation (same 8 KiB/partition size);
            # its buffer is recycled by the ThR stream after the preamble
            xT = wstr.tile([P, KTX, BL], BF16, tag="thr")
            for k in range(KTX):
                rings[k % 2].dma_start(out=xT[:, k, :], in_=xT_d[k * P:(k + 1) * P, :])
            for j in range(1, 3):
                txs = wpre.tile([P, KTX, P], BF16, tag="tx")
                nc.scalar.dma_start(out=txs, in_=Tx_d[j])
                wis = wpre.tile([P, KTX, P], BF16, tag="wi")
                nc.sync.dma_start(out=wis, in_=Wi_d[j])
                pre_slabs.append((txs, wis))

            taub = const.tile([P, JT], F32)
            nc.gpsimd.dma_start(out=taub, in_=taub_d[:])
            tbdt = const.tile([P, JT], F32)
            nc.gpsimd.dma_start(out=tbdt, in_=tbdt_d[:])
            htbdt = const.tile([P, JT], F32)
            nc.gpsimd.dma_start(out=htbdt, in_=htbdt_d[:])
            tb = const.tile([P, JT], F32)
            nc.gpsimd.dma_start(out=tb, in_=tb_d[:])
            htb = const.tile([P, JT], F32)
            nc.gpsimd.dma_start(out=htb, in_=htb_d[:])
            winbs = const.tile([P, JT], F32)
            nc.gpsimd.dma_start(out=winbs, in_=winbs_d[:])
            scl = const.tile([P, 2], F32)
            nc.gpsimd.dma_start(out=scl, in_=scl_d[:])

            hm = big.tile([P, KTH, BL], F32R, tag="hm")
            for k in range(KTH):
                nc.gpsimd.dma_start(out=hm[:, k, :], in_=hT_d[k * P:(k + 1) * P, :])

            # initial fp8 h state on the SWDGE queue with the rest of the
            # state (first consumed at step 0, ~60us in)
            h8_first = h8p.tile([P, KTH, BL], F8, tag="h8")
            for k in range(KTH):
                nc.gpsimd.dma_start(out=h8_first[:, k, :],
                                    in_=h8T_d[k * P:(k + 1) * P, :])

            # fp8 step weights: SBUF-resident, trickled in behind the
            # preamble slabs on both rings
            Wr8s = big.tile([P, JT, KTH, P], F8, tag="wr8")
            Th8s = big.tile([P, JT, KTH, P], F8, tag="th8")

            x_tau_s = big.tile([P, JT, BL], BF16, tag="xtau")
            ic_s = big.tile([P, JT, BL], BF16, tag="ic")

            # pipelined PSUM preloads of the scaled loop-invariants into
            # both accumulation banks (chains then run with start=False)
            pend = {}

            def preload_pt(s, j):
                xt_pt = ps.tile([P, BL], F32, tag="pt")
                nc.scalar.activation(xt_pt, x_tau_s[:, j, :], AF.Copy)
                ic_pr = ps.tile([P, BL], F32, tag="pr")
                nc.scalar.activation(ic_pr, ic_s[:, j, :], AF.Copy)
                pend[(s, j)] = (xt_pt, ic_pr)

            def preamble_j(j):
                if j < 3:
                    txs, wis = pre_slabs[j]
                else:
                    txs = wpre.tile([P, KTX, P], BF16, tag="tx")
                    nc.scalar.dma_start(out=txs, in_=Tx_d[j])
                    wis = wpre.tile([P, KTX, P], BF16, tag="wi")
                    nc.sync.dma_start(out=wis, in_=Wi_d[j])
                # trickle the resident fp8 weights in behind the preamble
                nc.scalar.dma_start(out=Wr8s[:, j, :, :], in_=Wr8_d[j])
                nc.sync.dma_start(out=Th8s[:, j, :, :], in_=Th8_d[j])
                pt = ps.tile([P, BL], F32, tag="pt")
                for k in range(KTX):
                    nc.tensor.matmul(pt, txs[:, k, :], xT[:, k, :],
                                     start=(k == 0), stop=(k == KTX - 1))
                nc.scalar.activation(x_tau_s[:, j, :], pt, AF.Copy)
                pr = ps.tile([P, BL], F32, tag="pr")
                for k in range(KTX):
                    nc.tensor.matmul(pr, wis[:, k, :], xT[:, k, :],
                                     start=(k == 0), stop=(k == KTX - 1))
                nc.scalar.activation(ic_s[:, j, :], pr, AF.Identity,
                                     bias=winbs[:, j:j + 1])

            def step_j(s, j, h8c, h8n, thr_slabs, emit_next=True):
                last = s == NUM_STEPS - 1
                pt, pr = pend.pop((s, j))
                if emit_next:
                    nj, ns = (j + 1, s) if j < JT - 1 else (0, s + 1)
                    if ns < NUM_STEPS:
                        preload_pt(ns, nj)

                # tau-path matmul chain, accumulating onto the preload
                if last:
                    thr = thr_slabs[j]
                    for k in range(KTH):
                        nc.tensor.matmul(pt, thr[:, k, :], hm[:, k, :],
                                         start=False, stop=(k == KTH - 1),
                                         skip_group_check=True)
                else:
                    for kp in range(KP):
                        nc.tensor.matmul(pt, Th8s[:, j, 2 * kp:2 * kp + 2, :],
                                         h8c[:, 2 * kp:2 * kp + 2, :],
                                         start=False, stop=(kp == KP - 1),
                                         perf_mode=DR, skip_group_check=True)
                # rec-path matmul chain, accumulating onto the preload
                for kp in range(KP):
                    nc.tensor.matmul(pr, Wr8s[:, j, 2 * kp:2 * kp + 2, :],
                                     h8c[:, 2 * kp:2 * kp + 2, :],
                                     start=False, stop=(kp == KP - 1),
                                     perf_mode=DR, skip_group_check=True)

                sig = sc.tile([P, BL], F32 if last else BF16, tag="sig")
                nc.scalar.activation(sig, pt, AF.Sigmoid,
                                     bias=taub[:, j:j + 1], scale=scl[:, 0:1])
                T_ = sc.tile([P, BL], F32, tag="T32")
                nc.scalar.activation(T_, sig, AF.Identity,
                                     bias=htbdt[:, j:j + 1], scale=tbdt[:, j:j + 1])
                if last:
                    tau32 = sc.tile([P, BL], F32, tag="tau32", bufs=1)
                    nc.scalar.activation(tau32, sig, AF.Identity,
                                         bias=htb[:, j:j + 1], scale=tb[:, j:j + 1])

                a_ = sc.tile([P, BL], BF16, tag="a")
                nc.scalar.activation(a_, pr, AF.Tanh, scale=scl[:, 1:2])
                q_ = sc.tile([P, BL], F32, tag="q", bufs=1)
                nc.vector.reciprocal_approx_fast(out=q_, in_=T_)
                d_ = sc.tile([P, BL], BF16, tag="d")
                nc.vector.tensor_tensor(out=d_, in0=a_, in1=hm[:, j, :],
                                        op=ALU.subtract)
                u_ = sc.tile([P, BL], BF16, tag="u")
                nc.vector.tensor_tensor(out=u_, in0=d_, in1=q_, op=ALU.mult)
                if last:
                    hfin = sc.tile([P, BL], F32, tag="hfin", bufs=1)
                    nc.vector.tensor_tensor(out=hfin, in0=u_, in1=hm[:, j, :],
                                            op=ALU.add)
                    # outputs trickle out on the SWDGE queue; the last four
                    # tiles ride the HWDGE rings (idle once ThR drains) so
                    # the SWDGE backlog never outlives the compute
                    if j >= JT - 4:
                        rings[j % 2].dma_start(out=hout_d[j * P:(j + 1) * P, :],
                                               in_=hfin)
                        rings[(j + 1) % 2].dma_start(
                            out=tauout_d[j * P:(j + 1) * P, :], in_=tau32)
                    else:
                        nc.gpsimd.dma_start(out=hout_d[j * P:(j + 1) * P, :], in_=hfin)
                        nc.gpsimd.dma_start(out=tauout_d[j * P:(j + 1) * P, :], in_=tau32)
                else:
                    nc.vector.tensor_tensor(out=hm[:, j, :], in0=u_,
                                            in1=hm[:, j, :], op=ALU.add)
                    nc.vector.tensor_copy(out=h8n[:, j, :], in_=hm[:, j, :])

            for j in range(JT):
                preamble_j(j)
            preload_pt(0, 0)

            h8c = h8_first
            thr_slabs = {}
            for s in range(NUM_STEPS):
                last = s == NUM_STEPS - 1
                if s == NUM_STEPS - 2:
                    # stream the f32r tau weights for the last step; the
                    # rings are otherwise idle once the preamble is done
                    for j in range(JT):
                        thr = wstr.tile([P, KTH, P], F32R, tag="thr")
                        rings[j % 2].dma_start(out=thr, in_=ThR_d[j])
                        thr_slabs[j] = thr
                h8n = None
                if not last:
                    h8n = h8p.tile([P, KTH, BL], F8, tag="h8")
                for j in range(JT):
                    step_j(s, j, h8c, h8n, thr_slabs)
                h8c = h8n
    nc.finalize()
    return nc


_NC_CACHE = None


def _get_nc():
    global _NC_CACHE
    if _NC_CACHE is None:
        _NC_CACHE = _build()
    return _NC_CACHE


def _prep_w(W):
    """W [J, K] row-major -> [jt, p, kt, c] with element [jt,p,kt,c] = W[jt*P+c, kt*P+p]."""
    J, K = W.shape
    ktn = K // P
    jtn = J // P
    Bv = np.ascontiguousarray(W.T).reshape(ktn, P, jtn, P)
    return np.ascontiguousarray(Bv.transpose(2, 1, 0, 3))


def _prep_vec(v):
    """[H] -> [P, JT] with col j = v[j*128:(j+1)*128]."""
    return np.ascontiguousarray(np.asarray(v, np.float32).reshape(JT, P).T)


def _pow2_scale(W):
    """Power-of-2 scale putting max|W| into (64, 128] for e4m3 quantization."""
    m = float(np.abs(W).max())
    if m == 0.0:
        return 1.0
    return float(2.0 ** np.floor(np.log2(128.0 / m)))


def kernel(x, hidden, W_rec, W_in_w, W_in_b, tau_base, tau_adapt_w, tau_adapt_b):
    global LAST_EXEC_NS
    x = np.asarray(x, np.float32)
    hidden = np.asarray(hidden, np.float32)
    W_rec = np.asarray(W_rec, np.float32)
    W_in_w = np.asarray(W_in_w, np.float32)
    W_in_b = np.asarray(W_in_b, np.float32)
    tau_base = np.asarray(tau_base, np.float32)
    tau_adapt_w = np.asarray(tau_adapt_w, np.float32)
    tau_adapt_b = np.asarray(tau_adapt_b, np.float32)

    Tx = tau_adapt_w[:, :I]
    Th = tau_adapt_w[:, I:]
    s_rec = _pow2_scale(W_rec)
    s_tau = _pow2_scale(Th)

    scl = np.empty((P, 2), np.float32)
    scl[:, 0] = 1.0 / s_tau
    scl[:, 1] = 1.0 / s_rec

    shared = {
        "Wi": _prep_w(W_in_w * s_rec).astype(NP_BF16),
        "Tx": _prep_w(Tx * s_tau).astype(NP_BF16),
        "Wr8": _prep_w(W_rec * s_rec).astype(NP_F8),
        "Th8": _prep_w(Th * s_tau).astype(NP_F8),
        "ThR": _prep_w(Th * s_tau),
        "taub": _prep_vec(tau_adapt_b),
        "tbdt": _prep_vec(tau_base / DT),
        "htbdt": _prep_vec(0.5 * tau_base / DT),
        "tb": _prep_vec(tau_base),
        "htb": _prep_vec(0.5 * tau_base),
        "winbs": _prep_vec(W_in_b * s_rec),
        "scl": scl,
    }
    in_maps = []
    for c in range(NCORES):
        sl = slice(c * BL, (c + 1) * BL)
        xTc = np.ascontiguousarray(x[sl].T)
        hTc = np.ascontiguousarray(hidden[sl].T)
        in_maps.append(dict(shared,
                            xT=xTc.astype(NP_BF16),
                            hT=hTc,
                            h8T=hTc.astype(NP_F8)))

    nc = _get_nc()
    trace = bool(os.environ.get("BASS_TRACE"))
    res = None
    for attempt in range(3):
        try:
            res = run_bass_kernel_spmd(nc, in_maps, list(range(NCORES)), trace=trace)
            break
        except Exception:
            # transient device errors (NRT unrecoverable) clear on retry
            # after the runtime resets the core
            if attempt == 2:
                raise

    if trace:
        LAST_EXEC_NS = res.exec_time_ns

    h_out = np.concatenate(
        [np.ascontiguousarray(res.results[c]["hout"].T) for c in range(NCORES)], axis=0)
    tau_out = np.concatenate(
        [np.ascontiguousarray(res.results[c]["tauout"].T) for c in range(NCORES)], axis=0)
    return h_out, tau_out
